# revision 10
# baseline (speedup 1.0000x reference)
"""Trainium2 Bass kernel for the three-GEU (text/video/audio) embedding model.

Strategy (8 NeuronCores, one chip):
  - Tensor-parallel column sharding: core c owns output columns [512c, 512(c+1))
    of every linear; it reads only its 1/8 slice of each weight matrix.
  - Gating fusion (host): g = (x@W.T+b)@Wg.T+bg == x@(Wg@W).T + (bg+Wg@b),
    so the gating GEMM reads the same gathered activations as the value GEMM.
    This removes the mid-kernel h-transpose + AllGather of the baseline and
    shrinks the audio gating weight from [4096,4096] to [4096,1024].
  - Preprocessing (text max-pool over L, audio ragged masked-mean over T) is
    sharded over the feature dim, computed in transposed layout, and an
    AllGather assembles the full [K, B] activations every core needs as the
    matmul stationary operand. This is the FIRST collective: it absorbs the
    inter-core launch skew while the weight stream saturates DMA.
  - Per embed: h = GEMM(x, Wslice), g = GEMM(x, Wfused_slice), y = h*sig(g),
    partial sum(y^2); one tiny AllReduce of the three norm partials, then
    reciprocal-sqrt scaling on-device.
  - fp16 operands into the PE (fp32 PSUM accumulation), fp32 outputs.
  - Video's GEMMs depend only on local inputs, so they run before the
    AllGather completes; PE order: audio matvecs, video h+g, text h+g,
    audio h+g (weights stream in that consumption order).
"""

import numpy as np

B = 64
L = 30
D = 4096
DA = 1024
T = 128
NCORES = 8
S = D // NCORES     # 512: per-core output shard of D
SA = DA // NCORES   # 128: per-core shard of Da
KD = D // 128       # 32 k-tiles over D
KA = DA // 128      # 8 k-tiles over Da
CH = 16             # k-tiles per weight DMA chunk (16 * 128 * 512 * 2B = 2 MiB)

_STATE: dict = {}


def _build():
    from contextlib import ExitStack

    import concourse.bass as bass
    import concourse.tile as tile
    from concourse import bacc, mybir

    fp16 = mybir.dt.float16
    f32 = mybir.dt.float32
    AX = mybir.AxisListType
    ALU = mybir.AluOpType
    ACTF = mybir.ActivationFunctionType

    nc = bacc.Bacc(
        "TRN2",
        target_bir_lowering=False,
        debug=False,
        enable_asserts=False,
        num_devices=NCORES,
    )
    RG = [list(range(NCORES))]

    # --- kernel I/O (per-core shards, staged by the host wrapper) ---
    # Weights arrive pre-tiled [n_chunks, 128, CH, S] so each chunk DMA is
    # fully contiguous per partition (16 KB).
    w_in = {}
    for name, kk in [("wv", D), ("wgv", D), ("wa", DA), ("wga", DA),
                     ("wt", D), ("wgt", D)]:
        nkt = kk // 128
        nch = max(1, nkt // CH)
        w_in[name] = nc.dram_tensor(
            name, [nch, 128, (nkt // nch) * S], fp16, kind="ExternalInput")
    textT = nc.dram_tensor("textT", [S, B, L], fp16, kind="ExternalInput")
    audioT = nc.dram_tensor("audioT", [T, B, SA], fp16, kind="ExternalInput")
    vT_d = nc.dram_tensor("vT", [128, KD, B], fp16, kind="ExternalInput")
    maskT_d = nc.dram_tensor("maskT", [T, B], fp16, kind="ExternalInput")
    biases_d = nc.dram_tensor("biases", [1, 6 * S], fp16, kind="ExternalInput")
    EMBEDS = ("text", "video", "audio")
    out_d = {
        e: nc.dram_tensor(f"out_{e}", [B, S], f32, kind="ExternalOutput")
        for e in EMBEDS
    }

    BIAS_IDX = {("text", 1): 0, ("text", 2): 1, ("video", 1): 2,
                ("video", 2): 3, ("audio", 1): 4, ("audio", 2): 5}

    with ExitStack() as ctx:
        tc = ctx.enter_context(tile.TileContext(nc))

        persist = ctx.enter_context(tc.tile_pool(name="persist", bufs=1))
        work = ctx.enter_context(tc.tile_pool(name="work", bufs=2))
        psum = ctx.enter_context(tc.tile_pool(name="psum", bufs=2, space="PSUM"))
        dram = ctx.enter_context(tc.tile_pool(name="dram", bufs=1, space="DRAM"))

        # ---- persistent SBUF tiles ----
        # All weights live in SBUF for the whole kernel (18 MB of the 26 MB
        # budget) so the PE never waits on a buffer-rotation dependency.
        w_sb = {}
        for name in ("wv", "wgv", "wa", "wga"):
            nch, _, csz = w_in[name].shape
            w_sb[name] = persist.tile([128, nch, csz], fp16, name=f"w_{name}")
        wpool = ctx.enter_context(tc.tile_pool(name="wstream", bufs=3))
        actsT = persist.tile([128, KD, B], fp16)      # gathered text k-tiles
        actsA = persist.tile([128, KA, B], fp16)      # gathered audio k-tiles
        au_sb = persist.tile([T, B, SA], fp16)        # audio shard, [t, b, c']
        vt_sb = persist.tile([128, KD, B], fp16)      # video.T k-tiles
        msk_sb = persist.tile([T, B], fp16)           # mask/nf, transposed
        bias_sb = persist.tile([1, 6, S], fp16)
        ones_sb = persist.tile([1, B], fp16)
        stg = persist.tile([128, 5, B], fp16)         # AG staging: 4 text + 1 audio k-tile rows
        nsq = persist.tile([B, 4], f32)               # partial sum(y^2) per embed
        nsqg = persist.tile([B, 3], f32)              # AllReduce result
        nrm = persist.tile([B, 3], f32)
        rcp = persist.tile([B, 3], f32)
        h16 = {e: persist.tile([B, S], fp16, name=f"h16_{e}") for e in EMBEDS}
        y_sb = {e: persist.tile([B, S], f32, name=f"y_{e}") for e in EMBEDS}

        # ---- constants ----
        nc.vector.memset(ones_sb[:], 1.0)
        nc.vector.memset(nsq[:], 0.0)

        # ---- input DMAs, all on the two HWDGE queues. The text tiles go
        # FIRST (bufs=4, no rotation dependency): the max-pool gates the
        # first collective's trigger, and the barrier that precedes the
        # first collective tracks the slowest core's trigger. ----
        t_view = textT.ap().rearrange("(n p) b l -> n p b l", p=128)
        tx = []
        for i in range(4):
            tx.append(work.tile([128, B, L], fp16, name="tx", bufs=4))
        nc.sync.dma_start(tx[0][:], t_view[0])
        nc.scalar.dma_start(tx[1][:], t_view[1])
        nc.sync.dma_start(au_sb[:], audioT.ap())
        nc.scalar.dma_start(bias_sb[0:1, :, :], biases_d.ap())
        nc.scalar.dma_start(vt_sb[:], vT_d.ap())
        nc.sync.dma_start(msk_sb[:], maskT_d.ap())
        nc.sync.dma_start(tx[2][:], t_view[2])
        nc.scalar.dma_start(tx[3][:], t_view[3])

        # ---- weight stream: all chunks up-front, in PE consumption order,
        # alternating between the two HWDGE queues ----
        hwdge = [nc.sync, nc.scalar]
        qi = 0
        for name in ("wv", "wgv"):
            nch = w_in[name].shape[0]
            for ch in range(nch):
                hwdge[qi % 2].dma_start(
                    w_sb[name][:, ch, :], w_in[name].ap()[ch])
                qi += 1

        def emit_audio_weight_dmas():
            for name in ("wa", "wga"):
                nc.sync.dma_start(w_sb[name][:, 0, :], w_in[name].ap()[0])

        # ---- text max-pool over L (sharded over d) -> stg[:, 0:4, :] ----
        for i in range(4):
            nc.vector.reduce_max(stg[:, i, :], tx[i][:], AX.X)

        # ---- audio ragged masked-mean (sharded over Da): 64 PE matvecs ----
        aT_ps = psum.tile([SA, B], f32, bufs=1)
        for b in range(B):
            nc.tensor.matmul(
                aT_ps[:, b:b + 1], au_sb[:, b, :], msk_sb[:, b:b + 1],
                start=True, stop=True)
        nc.vector.tensor_copy(stg[:, 4, :], aT_ps[:])

        # ---- AllGather the preprocessed activations, split in two: the
        # text gather (64 KB) fires as soon as the max-pool is done and is
        # the first collective; the audio gather (16 KB) runs behind it on
        # the CC stream, hiding under the text GEMMs. ----
        agt_in = dram.tile([128, 4 * B], fp16)
        agt_out = dram.tile([128 * NCORES, 4 * B], fp16, addr_space="Shared")
        aga_in = dram.tile([128, B], fp16)
        aga_out = dram.tile([128 * NCORES, B], fp16, addr_space="Shared")
        nc.gpsimd.dma_start(agt_in[:], stg[:, 0:4, :])
        nc.gpsimd.collective_compute(
            "AllGather", ALU.bypass, replica_groups=RG,
            ins=[agt_in.opt()], outs=[agt_out.opt()])
        nc.gpsimd.dma_start(aga_in[:], stg[:, 4, :])
        nc.gpsimd.collective_compute(
            "AllGather", ALU.bypass, replica_groups=RG,
            ins=[aga_in.opt()], outs=[aga_out.opt()])
        # text reload on scalar: it waits for the gather, and the only
        # thing it delays on that queue is wt2, which still lands before the
        # PE reaches it
        nc.scalar.dma_start(
            actsT.rearrange("p (r j) b -> p r j b", j=4),
            agt_out.rearrange("(r p) (j b) -> p r j b", p=128, b=B))
        nc.gpsimd.dma_start(
            actsA[:], aga_out.rearrange("(r p) b -> p r b", p=128))

        # lhsT accessors (stationary [128, B] k-tiles, transposed activations)
        def lhs_text(k):
            return actsT[:, k, :]

        def lhs_audio(k):
            return actsA[:, k, :]

        def lhs_video(k):
            return vt_sb[:, k, :]

        wq = [0]

        def gemm(out_ps, wname, n_kt, lhs_fn, bias_idx):
            # bias as a K=1 matmul row; also opens the accumulation group
            nc.tensor.matmul(out_ps[:], ones_sb[:], bias_sb[:, bias_idx, :],
                             start=True, stop=False)
            nch = w_in[wname].shape[0]
            cnt = n_kt // nch
            stream = wname not in w_sb
            if not stream:
                wview = w_sb[wname].rearrange("p c (a n) -> p c a n", n=S)
            for ch in range(nch):
                if stream:
                    wc = wpool.tile([128, cnt, S], fp16, name="wchunk",
                                    tag="wchunk")
                    hwdge[wq[0] % 2].dma_start(
                        wc[:], w_in[wname].ap()[ch].rearrange(
                            "p (a n) -> p a n", n=S))
                    wq[0] += 1
                for a in range(cnt):
                    k = ch * cnt + a
                    wtile = wc[:, a, :] if stream else wview[:, ch, a, :]
                    nc.tensor.matmul(out_ps[:], lhs_fn(k), wtile,
                                     start=False, stop=(k == n_kt - 1))

        # ---- the six GEMMs + GLU/partial-norm per embed ----
        # (video first: its inputs are local, so it runs during the skew
        # window while AG1 is still in flight)
        PLAN = [("video", "wv", "wgv", KD, lhs_video),
                ("text", "wt", "wgt", KD, lhs_text),
                ("audio", "wa", "wga", KA, lhs_audio)]
        EIDX = {e: i for i, e in enumerate(EMBEDS)}
        for e, wn1, wn2, nkt, lf in PLAN:
            if e == "audio":
                emit_audio_weight_dmas()
            ei = EIDX[e]
            h_ps = psum.tile([B, S], f32, name="h_ps", tag="h_ps")
            gemm(h_ps, wn1, nkt, lf, BIAS_IDX[(e, 1)])
            nc.vector.tensor_copy(h16[e][:], h_ps[:])
            g_ps = psum.tile([B, S], f32, name="g_ps", tag="g_ps")
            gemm(g_ps, wn2, nkt, lf, BIAS_IDX[(e, 2)])
            sg16 = work.tile([B, S], fp16, name="sg16", tag="sg16")
            nc.scalar.activation(sg16[:], g_ps[:], ACTF.Sigmoid)
            nc.vector.tensor_mul(y_sb[e][:], h16[e][:], sg16[:])
            # square + free-dim reduce fused on the Act engine (Square is in
            # the sigmoid table set, so no activation-table reload)
            ysq = work.tile([B, S], fp16, name="ysq", tag="ysq")
            nc.scalar.activation(ysq[:], y_sb[e][:], ACTF.Square,
                                 accum_out=nsq[:, ei:ei + 1])

        # ---- AllReduce norm partials; normalize; write outputs ----
        ar_in = dram.tile([B, 3], f32)
        ar_out = dram.tile([B, 3], f32, addr_space="Shared")
        nc.scalar.dma_start(ar_in[:], nsq[:, 0:3])
        nc.gpsimd.collective_compute(
            "AllReduce", ALU.add, replica_groups=RG,
            ins=[ar_in.opt()], outs=[ar_out.opt()])
        nc.scalar.dma_start(nsqg[:], ar_out[:])
        nc.scalar.sqrt(nrm[:], nsqg[:])
        nc.vector.tensor_scalar_max(nrm[:], nrm[:], 1e-12)
        nc.vector.reciprocal(rcp[:], nrm[:])
        oq = [nc.sync, nc.scalar, nc.gpsimd]
        for ei, e in enumerate(EMBEDS):
            yo = work.tile([B, S], f32, name="yo", tag="yo")
            nc.vector.tensor_scalar_mul(yo[:], y_sb[e][:],
                                        rcp[:, ei:ei + 1])
            oq[ei].dma_start(out_d[e].ap(), yo[:])

    nc.compile()
    return nc


def _get_nc():
    if "nc" not in _STATE:
        _STATE["nc"] = _build()
    return _STATE["nc"]


def _prep_inputs(text, video, audio_feats, Wt, bt, Wgt, bgt, Wv, bv, Wgv, bgv,
                 Wa, ba, Wga, bga, nframes, raw_audio_len):
    """Fuse gating weights, shard + transpose + fp16-cast into per-core maps."""
    f16 = np.float16
    text = np.asarray(text, dtype=np.float32)
    video = np.asarray(video, dtype=np.float32)
    audio = np.asarray(audio_feats, dtype=np.float32)

    Wt = np.asarray(Wt, dtype=np.float32)
    Wgt = np.asarray(Wgt, dtype=np.float32)
    Wv = np.asarray(Wv, dtype=np.float32)
    Wgv = np.asarray(Wgv, dtype=np.float32)
    Wa = np.asarray(Wa, dtype=np.float32)
    Wga = np.asarray(Wga, dtype=np.float32)
    bt = np.asarray(bt, dtype=np.float32)
    bgt = np.asarray(bgt, dtype=np.float32)
    bv = np.asarray(bv, dtype=np.float32)
    bgv = np.asarray(bgv, dtype=np.float32)
    ba = np.asarray(ba, dtype=np.float32)
    bga = np.asarray(bga, dtype=np.float32)

    # gating fusion: g = x @ (Wg@W).T + (bg + Wg@b)
    Wgt_f = Wgt @ Wt
    bgt_f = bgt + Wgt @ bt
    Wgv_f = Wgv @ Wv
    bgv_f = bgv + Wgv @ bv
    Wga_f = Wga @ Wa                     # [D, Da]
    bga_f = bga + Wga @ ba

    ratio = int(round(float(np.asarray(raw_audio_len)) / T))
    nf = np.maximum(
        1, (np.asarray(nframes).astype(np.float32) / ratio).astype(np.int32))
    mask = (np.arange(T)[None, :] < nf[:, None]).astype(np.float32)
    mask = mask / nf[:, None].astype(np.float32)          # [B, T] mask/nf
    maskT = np.ascontiguousarray(mask.T).astype(f16)      # [T, B]

    # video.T pre-tiled to [128, KD, B] (partition-contiguous k-tiles)
    vT = np.ascontiguousarray(
        video.T.reshape(KD, 128, B).transpose(1, 0, 2)).astype(f16)

    def wtile(W, sl):
        """W[sl].T [K, S] -> chunked [nch, 128, cnt*S], contiguous/partition."""
        wt = W[sl, :].T
        kk = wt.shape[0]
        nkt = kk // 128
        nch = max(1, nkt // CH)
        cnt = nkt // nch
        return np.ascontiguousarray(
            wt.reshape(nch, cnt, 128, S).transpose(0, 2, 1, 3)
            .reshape(nch, 128, cnt * S)).astype(f16)

    in_maps = []
    for c in range(NCORES):
        sl = slice(c * S, (c + 1) * S)
        sla = slice(c * SA, (c + 1) * SA)
        m = {
            "wt": wtile(Wt, sl),
            "wgt": wtile(Wgt_f, sl),
            "wv": wtile(Wv, sl),
            "wgv": wtile(Wgv_f, sl),
            "wga": wtile(Wga_f, sl),
            "wa": wtile(Wa, sl),
            "textT": np.ascontiguousarray(
                text[:, :, sl].transpose(2, 0, 1)).astype(f16),
            "audioT": np.ascontiguousarray(
                audio[:, sla, :].transpose(2, 0, 1)).astype(f16),
            "vT": vT,
            "maskT": maskT,
            "biases": np.stack([
                b[sl] for b in (bt, bgt_f, bv, bgv_f, ba, bga_f)
            ]).reshape(1, -1).astype(f16),
        }
        in_maps.append(m)
    return in_maps


def kernel(text, video, audio_feats, Wt, bt, Wgt, bgt, Wv, bv, Wgv, bgv,
           Wa, ba, Wga, bga, nframes, raw_audio_len):
    from concourse.bass_utils import run_bass_kernel_spmd

    nc = _get_nc()
    in_maps = _prep_inputs(text, video, audio_feats, Wt, bt, Wgt, bgt,
                           Wv, bv, Wgv, bgv, Wa, ba, Wga, bga,
                           nframes, raw_audio_len)
    res = run_bass_kernel_spmd(nc, in_maps, list(range(NCORES)))
    _STATE["last_results"] = res
    outs = []
    for e in ("text", "video", "audio"):
        outs.append(np.concatenate(
            [res.results[c][f"out_{e}"] for c in range(NCORES)], axis=1))
    return tuple(outs)
